# revision 6
# baseline (speedup 1.0000x reference)
"""Trainium2 Bass kernel for the reference MultiHeadAttention module.

Problem: B=32, T=512, D=1024, H=16, HD=64.

Reference semantics (keys index rows, softmax over query axis s, no scale):
    h  = x @ Wi + bi
    k/q/v = per-head h @ W{k,q,v}[h] + b
    wei[b,h,t,s] = k[b,h,t,:] . q[b,h,s,:]
    wei masked to s <= t, softmax over s
    out = (wei @ v) concat-heads @ Wo + bo

Design (fused, spill-free; ~1.65x the speed of the spill-based baseline;
v GEMM fully f32r -- no LDWEIGHTS, no bf16 x copy, better precision):
  * Wi is folded into the per-head QKV weights host-side
    (Wq' = Wi @ Wq_flat etc), eliminating the in_proj GEMM and the h
    activation entirely: 5 big GEMMs -> 4.
  * Fully fused per-batch pipeline: for each 512-token batch, q^T/k^T/v
    are computed straight from x^T and kept in SBUF; attention and
    out_proj follow immediately.  No DRAM scratch spills (saves ~48 MiB
    of HBM traffic per core vs the baseline).
  * v / P(=exp wei) / out-proj path AND the q/k tiles feeding the QK^T
    matmul in bf16 (measured end-to-end rel err 1.02e-2 on HW vs the
    2e-2 gate; the q/k GEMMs themselves stay f32r).  bf16 matmuls use a
    separate (hideable, FWL) weight load instead of the f32r
    self-loading matmul, and run full-rate at any moving width, so the
    S^T matmul computes exactly the causal column range.
  * Causal mask via 0/1 bf16 triangle multiply on the diagonal block
    AFTER exp (cheap DVE 4x mode) instead of additive -inf before exp.
  * Softmax denominator from a ones-column appended to v (row 64 of the
    PV matmul output); normalization = reciprocal + partition_broadcast
    + multiply in the attention tail per head.
  * Three-level software pipeline: attention head-pairs run a lag-1
    S^T/exp -> PV/norm pipeline, and between pairs the emitter interleaves
    chunks of independent PE work (previous batch's out_proj, next
    batch's stage1 GEMMs) so the PE never head-of-line blocks on the
    ACT exp stream.  Startup DMA is ordered so the first matmul waits
    only for x chunk 0 + the first wq column tile.

Sharding: data-parallel over batch; each of the 8 cores processes 4
batches with replicated weights; no collectives.  Host re-assembles the
full [32,512,1024] output and adds the bias constant.
"""

import sys

sys.path.insert(0, "/opt/trn_rl_repo")

import numpy as np

import concourse.bacc as bacc
import concourse.mybir as mybir
from concourse import bass_utils
from concourse.tile import TileContext

F32 = mybir.dt.float32
F32R = mybir.dt.float32r
BF16 = mybir.dt.bfloat16
AF = mybir.ActivationFunctionType

B, T, D, H, HD = 32, 512, 1024, 16, 64
NCORES = 8
BN = B // NCORES          # batches per core = 4
TOK = BN * T              # tokens per core = 2048
NKT = D // 128            # 8 contraction tiles

_CACHE = {}


def _build(with_qk_bias: bool, repeat: int = 1):
    nc = bacc.Bacc("TRN2", target_bir_lowering=False, debug=False,
                   num_devices=NCORES)

    xT = nc.dram_tensor("xT", [D, TOK], F32, kind="ExternalInput")
    wq = nc.dram_tensor("wq", [D, D], F32, kind="ExternalInput")
    wk = nc.dram_tensor("wk", [D, D], F32, kind="ExternalInput")
    wv = nc.dram_tensor("wv", [D, D], F32, kind="ExternalInput")
    wo = nc.dram_tensor("wo", [D, D], BF16, kind="ExternalInput")
    tri = nc.dram_tensor("tri", [128, 128], BF16, kind="ExternalInput")
    onesc = nc.dram_tensor("onesc", [128, H], BF16, kind="ExternalInput")
    if with_qk_bias:
        bq2 = nc.dram_tensor("bq2", [128, NKT], F32, kind="ExternalInput")
        bk2 = nc.dram_tensor("bk2", [128, NKT], F32, kind="ExternalInput")
    out = nc.dram_tensor("out", [TOK, D], F32, kind="ExternalOutput")

    with TileContext(nc) as tc:
      for _rep in range(repeat):
        with tc.tile_pool(name="w", bufs=1) as wpool, \
             tc.tile_pool(name="act", bufs=1) as apool, \
             tc.tile_pool(name="ps", bufs=1, space="PSUM") as pspool:
            # ---- weights + constants; DMA order = startup critical path.
            # q GEMM n-tile 0 needs wq cols [0:512) of every k-tile, so
            # issue those first, then x chunk 0, then the rest.
            wq_sb = [wpool.tile([128, D], F32R, tag=f"wq{k}", name=f"wq{k}")
                     for k in range(NKT)]
            wk_sb = [wpool.tile([128, D], F32R, tag=f"wk{k}", name=f"wk{k}")
                     for k in range(NKT)]
            wv_sb = [wpool.tile([128, D], F32R, tag=f"wv{k}", name=f"wv{k}")
                     for k in range(NKT)]
            wo_sb = [wpool.tile([128, D], BF16, tag=f"wo{k}", name=f"wo{k}")
                     for k in range(NKT)]
            # startup critical path: the first matmul needs x chunk 0 plus
            # only cols [0:128) of wq (n-tile 0), so land those first.
            xc0 = [apool.tile([128, 512], F32R, tag="xc", bufs=10,
                              name=f"xc{k}") for k in range(NKT)]
            for k in range(NKT):
                nc.sync.dma_start(xc0[k][:],
                                  xT[128 * k:128 * (k + 1), 0:512].bitcast(F32R))
                nc.sync.dma_start(wq_sb[k][:, 0:128],
                                  wq[128 * k:128 * (k + 1), 0:128].bitcast(F32R))
            for k in range(NKT):
                nc.sync.dma_start(wq_sb[k][:, 128:D],
                                  wq[128 * k:128 * (k + 1), 128:D].bitcast(F32R))
            for k in range(NKT):
                nc.sync.dma_start(wk_sb[k][:],
                                  wk[128 * k:128 * (k + 1), :].bitcast(F32R))
            tri_sb = wpool.tile([128, 128], BF16, tag="tri")
            nc.sync.dma_start(tri_sb[:], tri[:])
            ones16_sb = wpool.tile([128, H], BF16, tag="ones16")
            nc.sync.dma_start(ones16_sb[:], onesc[:])
            if with_qk_bias:
                bq_sb = wpool.tile([128, NKT], F32, tag="bq")
                bk_sb = wpool.tile([128, NKT], F32, tag="bk")
                nc.sync.dma_start(bq_sb[:], bq2[:])
                nc.sync.dma_start(bk_sb[:], bk2[:])
            for k in range(NKT):
                nc.sync.dma_start(wv_sb[k][:],
                                  wv[128 * k:128 * (k + 1), :].bitcast(F32R))

            def stage1(b, xc):
                """q^T/k^T (f32r, feature-major) + v_plus (bf16, token-major)
                for batch b, all SBUF-resident.  Returns ((qt, kt, vp), gen):
                tiles are allocated immediately; `gen` emits the GEMM work in
                chunks (one n-tile / tt-tile per next()) so callers can
                interleave it with other PE work."""
                qt = [apool.tile([128, 512], BF16, tag=f"qt{n}", bufs=1,
                                 name=f"qt{n}") for n in range(NKT)]
                kt = [apool.tile([128, 512], BF16, tag=f"kt{n}", bufs=1,
                                 name=f"kt{n}") for n in range(NKT)]
                vp = [apool.tile([128, H * 65], BF16, tag=f"vp{i}", bufs=1,
                                 name=f"vp{i}") for i in range(4)]

                def gen():
                    for w_sb, dst, bias in ((wq_sb, qt, "bq"), (wk_sb, kt, "bk")):
                        for n in range(NKT):
                            ph = pspool.tile([128, 512], F32, tag="ps", bufs=2)
                            for k in range(NKT):
                                nc.tensor.matmul(
                                    ph[:], w_sb[k][:, 128 * n:128 * (n + 1)],
                                    xc[k][:],
                                    start=(k == 0), stop=(k == NKT - 1))
                            if with_qk_bias:
                                bap = (bq_sb if bias == "bq" else bk_sb)[:, n:n + 1]
                                nc.vector.tensor_scalar_add(dst[n][:], ph[:], bap)
                            else:
                                # ACT evac keeps DVE free for the attention tail
                                nc.scalar.copy(dst[n][:], ph[:])
                            yield
                    # v GEMM fully f32r: xc slice self-loads as stationary,
                    # wv streams as f32r moving -- no LDWEIGHTS, no bf16 x copy
                    for tt in range(4):
                        v3 = vp[tt][:].rearrange("p (h e) -> p h e", e=65)
                        for nn in range(2):
                            pv = pspool.tile([128, 512], F32, tag="ps", bufs=2)
                            for k in range(NKT):
                                nc.tensor.matmul(
                                    pv[:], xc[k][:, 128 * tt:128 * (tt + 1)],
                                    wv_sb[k][:, 512 * nn:512 * (nn + 1)],
                                    start=(k == 0), stop=(k == NKT - 1))
                            nc.vector.tensor_copy(
                                v3[:, 8 * nn:8 * (nn + 1), 0:64],
                                pv[:].rearrange("p (h e) -> p h e", e=64))
                        nc.vector.tensor_copy(v3[:, :, 64], ones16_sb[:])
                        yield

                return (qt, kt, vp), gen()

            def attention(b, qt, kt, vp, filler=None):
                """16 heads of attention for batch b -> normalized oT (bf16).
                Lag-1 pipeline: S^T/exp of pair m before PV/norm of m-1.
                `filler` is a generator emitting one chunk of independent PE
                work (previous batch's out_proj) per pair, so the PE has
                runway while the ACT exp stream paces the attention chain."""
                oT = [apool.tile([128, 512], BF16, tag=f"oT{e}", bufs=2,
                                 name=f"oT{e}") for e in range(NKT)]

                def scores(m):
                    pts = {}
                    for i in range(4):
                        w0 = 128 * i          # valid t-cols are [w0, 512)
                        wd = 512 - w0
                        # both heads' S^T into one 2-bank psum tile; ONE exp
                        ps2 = pspool.tile([128, 1024], F32, tag="psS", bufs=2,
                                          name=f"ps{i}")
                        pt2 = apool.tile([128, 2 * wd], BF16, tag=f"pt{i}",
                                         bufs=(2 if i == 3 else 3),
                                         name=f"pt{i}")
                        for jj, j in enumerate((2 * m, 2 * m + 1)):
                            off = 64 * (j % 2)
                            nc.tensor.matmul(
                                ps2[:, 512 * jj:512 * jj + wd],
                                qt[m][off:off + 64, w0:w0 + 128],
                                kt[m][off:off + 64, w0:512],
                                start=True, stop=True)
                        pin = ps2[:].rearrange("p (u w) -> p u w", u=2)[:, :, 0:wd]
                        pout = pt2[:].rearrange("p (u w) -> p u w", u=2)
                        nc.scalar.activation(pout, pin, AF.Exp)
                        for jj, j in enumerate((2 * m, 2 * m + 1)):
                            pt = pt2[:, wd * jj:wd * (jj + 1)]
                            # causal mask: zero s>t on the diagonal block
                            nc.vector.tensor_mul(pt[:, 0:128], pt[:, 0:128],
                                                 tri_sb[:])
                            pts[(j, i)] = pt
                    return pts

                def pv_norm(m, pts):
                    for j in (2 * m, 2 * m + 1):
                        off = 64 * (j % 2)
                        po = pspool.tile([65, 512], F32, tag="po", bufs=2,
                                         name=f"po{j % 2}")
                        for i in range(4):
                            w0 = 128 * i
                            nc.tensor.matmul(
                                po[0:65, w0:512],
                                vp[i][:, 65 * j:65 * (j + 1)],
                                pts[(j, i)],
                                start=(i == 0), stop=(i == 3),
                                skip_group_check=True)
                        rs = apool.tile([1, 512], BF16, tag="rs", bufs=2,
                                        name="rs")
                        with nc.allow_low_precision(reason="f32r softmax recip"):
                            nc.vector.reciprocal(rs[:], po[64:65, :])
                        rb = apool.tile([64, 512], BF16, tag="rb", bufs=2,
                                        name="rb")
                        nc.gpsimd.partition_broadcast(rb[:], rs[:])
                        nc.vector.tensor_mul(oT[m][off:off + 64, :],
                                             po[0:64, :], rb[:])

                pend = None
                for m in range(H // 2):
                    pts = scores(m)
                    if pend is not None:
                        pv_norm(*pend)
                    if filler is not None:
                        next(filler, None)
                        next(filler, None)
                    pend = (m, pts)
                pv_norm(*pend)
                return oT

            def out_proj_chunks(b, oT):
                """Generator: one (tt, nn) chunk of batch b's out_proj per
                next() call; used as attention-filler for batch b+1."""
                r0 = 512 * b
                for tt in range(4):
                    for nn in range(2):
                        pf = pspool.tile([128, 512], F32, tag="ps", bufs=2)
                        for k in range(NKT):
                            # stationary oT tile is reused for both nn halves
                            nc.tensor.matmul(
                                pf[:], oT[k][:, 128 * tt:128 * (tt + 1)],
                                wo_sb[k][:, 512 * nn:512 * (nn + 1)],
                                start=(k == 0), stop=(k == NKT - 1))
                        os_ = apool.tile([128, 512], F32, tag="os", bufs=2,
                                         name=f"os{tt}{nn}")
                        nc.vector.tensor_copy(os_[:], pf[:])
                        nc.sync.dma_start(
                            out[r0 + 128 * tt:r0 + 128 * (tt + 1),
                                512 * nn:512 * (nn + 1)], os_[:])
                        yield

            import itertools

            def load_xc(b):
                xc = [apool.tile([128, 512], F32R, tag="xc", bufs=10,
                                 name=f"xc{k}") for k in range(NKT)]
                for k in range(NKT):
                    nc.sync.dma_start(
                        xc[k][:],
                        xT[128 * k:128 * (k + 1),
                           512 * b:512 * (b + 1)].bitcast(F32R))
                return xc

            # Software pipeline across batches.  attention(b)'s filler gets
            # one chunk of independent PE work per head-pair: first batch
            # b-1's out_proj chunks, then batch b+1's stage1 GEMM chunks
            # (for b=0 the latter is all there is).  Whatever attention
            # doesn't consume is drained right after, preserving emission
            # order for batch b+1.
            (h0, g0) = stage1(0, xc0)
            for _ in g0:  # bootstrap: batch 0's stage1 emitted serially
                pass
            handles = {0: h0}
            op_gen = None
            for b in range(BN):
                parts = []
                if op_gen is not None:
                    parts.append(op_gen)
                s1_gen = None
                if b + 1 < BN:
                    handles[b + 1], s1_gen = stage1(b + 1, load_xc(b + 1))
                    parts.append(s1_gen)
                if b == 0:
                    for k in range(NKT):
                        nc.sync.dma_start(wo_sb[k][:],
                                          wo[128 * k:128 * (k + 1), :])
                filler = itertools.chain(*parts) if parts else None
                qt, kt, vp = handles[b]
                oT = attention(b, qt, kt, vp, filler)
                for _ in (filler or ()):  # drain unfilled chunks
                    pass
                op_gen = out_proj_chunks(b, oT)
            for _ in op_gen:  # final batch's out_proj
                pass

    nc.compile()
    return nc


def _ensure_built(with_qk_bias: bool, repeat: int = 1):
    key = (with_qk_bias, repeat)
    if key not in _CACHE:
        _CACHE[key] = _build(with_qk_bias, repeat)
    return _CACHE[key]


def _prepare(x, Wi, bi, Wk, bk, Wq, bq, Wv, bv, Wo, bo):
    """Host-side prep: returns (in_maps, out_const, with_qk_bias)."""
    import ml_dtypes

    x = np.asarray(x, np.float32)
    Wi64 = np.asarray(Wi, np.float64)
    bi = np.asarray(bi, np.float32)
    Wo = np.asarray(Wo, np.float32)
    bo = np.asarray(bo, np.float32)

    # flatten head-stacked weights: col f = h*HD + e
    wq_f = np.asarray(Wq, np.float64).transpose(1, 0, 2).reshape(D, D)
    wk_f = np.asarray(Wk, np.float64).transpose(1, 0, 2).reshape(D, D)
    wv_f = np.asarray(Wv, np.float64).transpose(1, 0, 2).reshape(D, D)
    # fold Wi through the qkv projections (in_proj GEMM disappears)
    wq_p = np.ascontiguousarray((Wi64 @ wq_f).astype(np.float32))
    wk_p = np.ascontiguousarray((Wi64 @ wk_f).astype(np.float32))
    wv_p = (Wi64 @ wv_f).astype(np.float32)
    wo_b = Wo.astype(ml_dtypes.bfloat16)

    # fold bi through the qkv projections; fold bv through out_proj
    bq_fold = (bi @ wq_f + np.asarray(bq, np.float64).reshape(-1)).astype(np.float32)
    bk_fold = (bi @ wk_f + np.asarray(bk, np.float64).reshape(-1)).astype(np.float32)
    bv_fold = (bi @ wv_f + np.asarray(bv, np.float64).reshape(-1)).astype(np.float32)
    out_const = (bv_fold @ Wo + bo).astype(np.float32)  # added host-side

    with_qk_bias = bool(np.any(bq_fold) or np.any(bk_fold))

    tri01 = np.triu(np.ones((128, 128))).astype(ml_dtypes.bfloat16)
    onesc = np.ones((128, H), ml_dtypes.bfloat16)

    shared = {"wq": wq_p, "wk": wk_p, "wv": np.ascontiguousarray(wv_p),
              "wo": np.ascontiguousarray(wo_b), "tri": tri01, "onesc": onesc}
    if with_qk_bias:
        shared["bq2"] = np.ascontiguousarray(bq_fold.reshape(NKT, 128).T)
        shared["bk2"] = np.ascontiguousarray(bk_fold.reshape(NKT, 128).T)

    in_maps = []
    for c in range(NCORES):
        xs = x[BN * c:BN * (c + 1)].reshape(TOK, D)
        m = dict(shared)
        m["xT"] = np.ascontiguousarray(xs.T)
        in_maps.append(m)
    return in_maps, out_const, with_qk_bias


def kernel(x, Wi, bi, Wk, bk, Wq, bq, Wv, bv, Wo, bo):
    in_maps, out_const, with_qk_bias = _prepare(
        x, Wi, bi, Wk, bk, Wq, bq, Wv, bv, Wo, bo)
    nc = _ensure_built(with_qk_bias)
    res = bass_utils.run_bass_kernel_spmd(nc, in_maps, core_ids=list(range(NCORES)))
    outs = [res.results[c]["out"] for c in range(NCORES)]
    full = np.concatenate(outs, axis=0).reshape(B, T, D)
    full += out_const[None, None, :]
    return full

